# revision 14
# baseline (speedup 1.0000x reference)
"""Multi-head causal self-attention on 8 Trainium2 NeuronCores.

Sharding: 4-way data parallel over batch x 2-way tensor parallel over heads.
Core c handles batch c//2 and head group c%2 (8 of 16 heads). Partial
output projections (row-split Wo) are summed on the host; bias is added
on-device by the g=0 core (g=1 cores receive a zero bias).

v2 kernel: heads are processed in PAIRS. The score matmuls (contraction
dim = head_dim = 64) for the two heads of a pair are emitted back-to-back
with lhsT/rhs at base partitions 0 and 64, so the auto-derived
tile_position packs them into the top/bottom halves of the PE array and
they run concurrently (2x score throughput). Their outputs land in one
[128, 2048] PSUM region (head A cols 0:1024, head B 1024:2048) so a
single ACTIVATE computes exp for both heads (halves ACT call count).
P tiles are retained per 8-kt group and the ctx matmuls (V^T @ P, M=65
with a ones-column for softmax denominators) run chunk-outer with per-kt
narrowed column ranges (no zero-fill memsets); for the second q-half the
two kt-groups produce partial context sums combined in SBUF. Causal-diag
masks run on the otherwise-idle GPSIMD engine. QKV projection chains for
the second half and output-projection chunks are interleaved into the
attention emission as PE fill units so the tensor engine never idles
long enough for the HAM clock gate to re-throttle it to 1.2 GHz.
"""

import numpy as np

B, T, D = 4, 2048, 1024
HEADS = 16
N_CORES = 8
HPC = 8               # heads per core
HD = HPC * 64         # 512, per-core projection width
NT_D = D // 128       # 8 d-tiles
TB = 512              # t-block width for QKV streaming
NT_T = T // 128       # 16 t-tiles
HALF = 1024           # attention q-half width
CHUNK = 512
NPAIR = 4
KGRP = 8              # kt tiles per retained-P group

_NC = None


def _build():
    import concourse.tile as tile
    import concourse.mybir as mybir
    from concourse import bacc
    from contextlib import ExitStack
    from collections import deque

    F32 = mybir.dt.float32
    F32R = mybir.dt.float32r
    BF16 = mybir.dt.bfloat16
    EXP = mybir.ActivationFunctionType.Exp

    nc = bacc.Bacc("TRN2", target_bir_lowering=False, debug=False,
                   num_devices=N_CORES)

    xT_ext = nc.dram_tensor("xT", [D, T], BF16, kind="ExternalInput")
    wqT_ext = nc.dram_tensor("wqT", [D, HD], BF16, kind="ExternalInput")
    wkT_ext = nc.dram_tensor("wkT", [D, HD], BF16, kind="ExternalInput")
    wvT_ext = nc.dram_tensor("wvT", [D, HD], BF16, kind="ExternalInput")
    woT_ext = nc.dram_tensor("woT", [HD, D], BF16, kind="ExternalInput")
    bias_ext = nc.dram_tensor("bias", [1, D], F32, kind="ExternalInput")
    mask_ext = nc.dram_tensor("mask", [128, 128], BF16, kind="ExternalInput")
    out_ext = nc.dram_tensor("out", [T, D], F32, kind="ExternalOutput")
    import os as _os
    DBG = bool(_os.environ.get("KERNEL_DEBUG_TAPS"))
    if DBG:
        dbg_q = nc.dram_tensor("dbg_q", [128, T], BF16, kind="ExternalOutput")
        dbg_k = nc.dram_tensor("dbg_k", [128, T], BF16, kind="ExternalOutput")
        dbg_pt = nc.dram_tensor("dbg_pt", [128, 2 * HALF], BF16,
                                kind="ExternalOutput")
        dbg_lb = nc.dram_tensor("dbg_lb", [65, HALF], F32,
                                kind="ExternalOutput")
        dbg_ctx = nc.dram_tensor("dbg_ctx", [128, HALF], BF16,
                                 kind="ExternalOutput")

    with tile.TileContext(nc) as tc, ExitStack() as ctx:
        # ---- pools (PSUM: 4 + 2 + 2 = 8 banks) ------------------------
        wqkv_pool = ctx.enter_context(tc.tile_pool(name="wqkv", bufs=1))
        wo_pool = ctx.enter_context(tc.tile_pool(name="wo", bufs=1))
        qk_pool = ctx.enter_context(tc.tile_pool(name="qk", bufs=1))
        v_pool = ctx.enter_context(tc.tile_pool(name="v", bufs=1))
        xt_pool = ctx.enter_context(tc.tile_pool(name="xt", bufs=2))
        small = ctx.enter_context(tc.tile_pool(name="small", bufs=1))
        pt_pool = ctx.enter_context(tc.tile_pool(name="pt", bufs=1))
        ctxT_pool = ctx.enter_context(tc.tile_pool(name="ctxT", bufs=2))
        norm_pool = ctx.enter_context(tc.tile_pool(name="norm", bufs=2))
        lbuf_pool = ctx.enter_context(tc.tile_pool(name="lbuf", bufs=1))
        out_pool = ctx.enter_context(tc.tile_pool(name="outsb", bufs=3))
        s_ps_pool = ctx.enter_context(
            tc.tile_pool(name="sps", bufs=2, space="PSUM"))
        ctx_ps_pool = ctx.enter_context(
            tc.tile_pool(name="ctxps", bufs=2, space="PSUM"))
        mm_ps_pool = ctx.enter_context(
            tc.tile_pool(name="mmps", bufs=2, space="PSUM"))

        mask_sb = small.tile([128, 128], BF16, tag="mask")
        nc.sync.dma_start(mask_sb[:], mask_ext[:])
        bias_row = small.tile([1, D], F32, tag="biasrow")
        nc.sync.dma_start(bias_row[:], bias_ext[:])
        bias_bc = small.tile([128, D], F32, tag="biasbc")
        nc.gpsimd.partition_broadcast(bias_bc[:], bias_row[:])

        QT = [qk_pool.tile([128, T], BF16, tag=f"QT{i}", name=f"QT{i}")
              for i in range(NPAIR)]
        KT = [qk_pool.tile([128, T], BF16, tag=f"KT{i}", name=f"KT{i}")
              for i in range(NPAIR)]
        V = [v_pool.tile([128, HPC * 65], BF16, tag=f"V{i}", name=f"V{i}")
             for i in range(NT_T)]

        w_sb = {}

        def load_w(name, ext):
            t_ = wqkv_pool.tile([128, NT_D * HD], BF16, tag=f"w{name}",
                                name=f"w{name}")
            nc.sync.dma_start(
                t_[:].rearrange("p (dt c) -> p dt c", c=HD),
                ext[:].rearrange("(dt p) c -> p dt c", p=128))
            return [t_[:, dt * HD:(dt + 1) * HD] for dt in range(NT_D)]

        def load_x(tb):
            t_ = xt_pool.tile([128, NT_D * TB], BF16, tag=f"xt{tb % 2}",
                              name=f"xt{tb % 2}")
            nc.scalar.dma_start(
                t_[:].rearrange("p (dt c) -> p dt c", c=TB),
                xT_ext[:, tb * TB:(tb + 1) * TB]
                .rearrange("(dt p) c -> p dt c", p=128))
            return [t_[:, dt * TB:(dt + 1) * TB] for dt in range(NT_D)]

        def qk_chain(tb, xts, wname, dst, hdt):
            ps = mm_ps_pool.tile([128, TB], F32, tag="mm", name="mmps")
            for dt in range(NT_D):
                nc.tensor.matmul(
                    ps[:],
                    w_sb[wname][dt][:, hdt * 128:(hdt + 1) * 128],
                    xts[dt][:],
                    start=(dt == 0), stop=(dt == NT_D - 1))
            nc.vector.tensor_copy(
                dst[hdt][:, tb * TB:(tb + 1) * TB], ps[:])

        def v_chain(tb, xts, j):
            tt = tb * (TB // 128) + j
            ps = mm_ps_pool.tile([128, HD], F32, tag="mm", name="vps")
            for dt in range(NT_D):
                nc.tensor.matmul(
                    ps[:],
                    xts[dt][:, j * 128:(j + 1) * 128],
                    w_sb["v"][dt][:],
                    start=(dt == 0), stop=(dt == NT_D - 1))
            v3 = V[tt][:].rearrange("p (h c) -> p h c", c=65)
            nc.vector.memset(v3[:, :, 64:65], 1.0)
            nc.vector.tensor_copy(
                v3[:, :, 0:64],
                ps[:].rearrange("p (h c) -> p h c", c=64))

        def qkv_units(half):
            """Yield closures, each emitting one projection chain."""
            tbs = (0, 1) if half == 0 else (2, 3)
            if half == 0:
                w_sb["q"] = load_w("q", wqT_ext)
            xts_all = {tb: load_x(tb) for tb in tbs}
            if half == 0:
                w_sb["k"] = load_w("k", wkT_ext)
                w_sb["v"] = load_w("v", wvT_ext)
                wo_all = wo_pool.tile([128, NPAIR * D], BF16, tag="wo",
                                      name="wo")
                nc.sync.dma_start(
                    wo_all[:].rearrange("p (hdt c) -> p hdt c", c=D),
                    woT_ext[:].rearrange("(hdt p) c -> p hdt c", p=128))
                w_sb["o"] = [wo_all[:, hdt * D:(hdt + 1) * D]
                             for hdt in range(NPAIR)]
            q_units, kv_units = [], []
            for tb in tbs:
                for hdt in range(NPAIR):
                    q_units.append(
                        (lambda tb=tb, h=hdt:
                         qk_chain(tb, xts_all[tb], "q", QT, h)))
            for hdt in range(NPAIR):        # pair-major: KT[p] ready early
                for tb in tbs:
                    kv_units.append(
                        (lambda tb=tb, h=hdt:
                         qk_chain(tb, xts_all[tb], "k", KT, h)))
            for tb in tbs:
                for j in range(TB // 128):
                    kv_units.append(
                        (lambda tb=tb, j=j: v_chain(tb, xts_all[tb], j)))
            return deque(q_units), deque(kv_units)

        def scores_exp_pair(p, half, kt, pts):
            q0 = half * HALF
            off = max(0, kt * 128 - q0)
            pt = pt_pool.tile([128, 2 * HALF], BF16, tag=f"pt{kt % KGRP}",
                              name=f"pt{kt % KGRP}")
            pts[kt] = pt
            p3 = pt[:].rearrange("p (g q) -> p g q", q=HALF)
            for c in range(HALF // CHUNK):
                lo = c * CHUNK
                if lo + CHUNK <= off:
                    continue
                sub = max(0, off - lo)
                s = s_ps_pool.tile([128, 2 * CHUNK], F32, tag="s",
                                   name="sps")
                for h2 in range(2):
                    nc.tensor.matmul(
                        s[:, h2 * CHUNK + sub:(h2 + 1) * CHUNK],
                        KT[p][h2 * 64:h2 * 64 + 64,
                              kt * 128:(kt + 1) * 128],
                        QT[p][h2 * 64:h2 * 64 + 64,
                              q0 + lo + sub:q0 + lo + CHUNK],
                        start=True, stop=True)
                s3 = s[:].rearrange("p (g q) -> p g q", q=CHUNK)
                nc.scalar.activation(p3[:, :, lo + sub:lo + CHUNK],
                                     s3[:, :, sub:CHUNK], EXP, scale=0.125)
            if kt * 128 >= q0:
                for h2 in range(2):
                    nc.gpsimd.tensor_mul(
                        pt[:, h2 * HALF + off:h2 * HALF + off + 128],
                        pt[:, h2 * HALF + off:h2 * HALF + off + 128],
                        mask_sb[:])

        def ctx_unit(p, half, h2, c, pts, ctxT_h, lbuf, first):
            """Context-chain over the kt's present in pts for chunk c."""
            q0 = half * HALF
            h = 2 * p + h2
            lo = c * CHUNK
            kts = [kt for kt in sorted(pts)
                   if max(0, kt * 128 - q0) < lo + CHUNK]
            if not kts:
                return
            cps = ctx_ps_pool.tile([65, CHUNK], F32, tag="ctx", name="ctxps")
            for i, kt in enumerate(kts):
                sub = max(0, max(0, kt * 128 - q0) - lo)
                nc.tensor.matmul(
                    cps[:, sub:CHUNK],
                    V[kt][:, h * 65:(h + 1) * 65],
                    pts[kt][:, h2 * HALF + lo + sub:h2 * HALF + lo + CHUNK],
                    start=(i == 0), stop=(i == len(kts) - 1),
                    skip_group_check=True)
            dst = ctxT_h[p][h2 * 64:h2 * 64 + 64, lo:lo + CHUNK]
            dden = lbuf[p][h2 * 64:h2 * 64 + 1, lo:lo + CHUNK]
            if first:
                nc.vector.tensor_copy(dst, cps[0:64, :])
                nc.vector.tensor_copy(dden, cps[64:65, :])
            else:
                nc.vector.tensor_add(dst, dst, cps[0:64, :])
                nc.vector.tensor_add(dden, dden, cps[64:65, :])

        def norm_pair(ctxT_h, lbuf, p):
            for cc in range(HALF // CHUNK):
                sl = slice(cc * CHUNK, (cc + 1) * CHUNK)
                nc.vector.reciprocal(lbuf[p][:, sl], lbuf[p][:, sl])
                for h2 in range(2):
                    po = h2 * 64
                    stage = norm_pool.tile([1, CHUNK], BF16, tag="stage",
                                           name="stage")
                    nc.vector.tensor_copy(stage[:],
                                          lbuf[p][po:po + 1, sl])
                    bc = norm_pool.tile([128, CHUNK], BF16, tag="bc",
                                        name="bc")
                    nc.gpsimd.partition_broadcast(bc[:], stage[:])
                    nc.gpsimd.tensor_mul(
                        ctxT_h[p][po:po + 64, sl],
                        ctxT_h[p][po:po + 64, sl],
                        bc[po:po + 64, :])

        def outproj_units(half, ctxT_h):
            q0 = half * HALF
            units = []

            def one(tt, ob):
                ps = mm_ps_pool.tile([128, CHUNK], F32, tag="mm", name="opps")
                for p in range(NPAIR):
                    nc.tensor.matmul(
                        ps[:],
                        ctxT_h[p][:, tt * 128:(tt + 1) * 128],
                        w_sb["o"][p][:, ob * CHUNK:(ob + 1) * CHUNK],
                        start=(p == 0), stop=(p == NPAIR - 1))
                osb = out_pool.tile([128, CHUNK], F32, tag="osb", name="osb")
                nc.vector.tensor_add(
                    osb[:], ps[:], bias_bc[:, ob * CHUNK:(ob + 1) * CHUNK])
                nc.sync.dma_start(
                    out_ext[q0 + tt * 128:q0 + (tt + 1) * 128,
                            ob * CHUNK:(ob + 1) * CHUNK],
                    osb[:])

            for tt in range(HALF // 128):
                for ob in range(D // CHUNK):
                    units.append(lambda tt=tt, ob=ob: one(tt, ob))
            return deque(units)

        def attn_half(half, fill):
            q0 = half * HALF
            nkt = (q0 + HALF) // 128
            ctxT_h = [ctxT_pool.tile([128, HALF], BF16, tag=f"ctxT{i}",
                                     name=f"ctxT{i}")
                      for i in range(NPAIR)]
            lbuf = [lbuf_pool.tile([65, HALF], F32, tag=f"lb{j}",
                                   name=f"lb{j}")
                    for j in range(NPAIR)]
            for p in range(NPAIR):
                for g in range(nkt // KGRP):
                    pts = {}
                    for kt in range(g * KGRP, (g + 1) * KGRP):
                        scores_exp_pair(p, half, kt, pts)
                        if DBG and half == 0 and p == 0 and kt == 0:
                            nc.sync.dma_start(dbg_pt[:], pts[0][:])
                        if fill:
                            fill.popleft()()
                    for c in range(HALF // CHUNK):
                        for h2 in range(2):
                            ctx_unit(p, half, h2, c, pts, ctxT_h, lbuf,
                                     first=(g == 0))
                norm_pair(ctxT_h, lbuf, p)
            if DBG and half == 0:
                nc.sync.dma_start(dbg_lb[:], lbuf[0][:])
            if DBG and half == 0:
                nc.sync.dma_start(dbg_ctx[:], ctxT_h[0][:])
                nc.sync.dma_start(dbg_q[:], QT[0][:])
                nc.sync.dma_start(dbg_k[:], KT[0][:])
            return ctxT_h

        # ---------------- emission schedule ----------------------------
        q0u, kv0u = qkv_units(0)
        while q0u:
            q0u.popleft()()
        while kv0u:
            kv0u.popleft()()

        q1u, kv1u = qkv_units(1)        # emits x(h1) load DMAs up front
        ctxT_0 = attn_half(0, q1u)      # Q(h1) chains fill attn(h0)
        while q1u:
            q1u.popleft()()

        fill1 = kv1u                    # K/V(h1) chains fill attn(h1) grp 0
        ctxT_1_box = []

        def run_attn1():
            # outproj(h0) chunks join the fill queue once norm(h0) is done
            fill1.extend(outproj_units(0, ctxT_0))
            ctxT_1_box.append(attn_half(1, fill1))

        run_attn1()
        while fill1:
            fill1.popleft()()

        op1 = outproj_units(1, ctxT_1_box[0])
        while op1:
            op1.popleft()()

    nc.compile()
    return nc


def _get_nc():
    global _NC
    if _NC is None:
        _NC = _build()
    return _NC


def _make_in_maps(x, Wq, Wk, Wv, Wo, bo):
    import ml_dtypes
    mask = np.triu(np.ones((128, 128), dtype=np.float32)).astype(
        ml_dtypes.bfloat16)
    zero_bias = np.zeros((1, D), dtype=np.float32)
    xT = [np.ascontiguousarray(x[b].T) for b in range(B)]
    in_maps = []
    for c in range(N_CORES):
        b, g = c // 2, c % 2
        sl = slice(g * HD, (g + 1) * HD)
        in_maps.append({
            "xT": xT[b].astype(ml_dtypes.bfloat16),
            "wqT": np.ascontiguousarray(Wq[sl, :].T).astype(ml_dtypes.bfloat16),
            "wkT": np.ascontiguousarray(Wk[sl, :].T).astype(ml_dtypes.bfloat16),
            "wvT": np.ascontiguousarray(Wv[sl, :].T).astype(ml_dtypes.bfloat16),
            "woT": np.ascontiguousarray(Wo[:, sl].T).astype(ml_dtypes.bfloat16),
            "bias": bo.reshape(1, D) if g == 0 else zero_bias,
            "mask": mask,
        })
    return in_maps


def kernel(x, Wq, Wk, Wv, Wo, bo):
    from concourse.bass_utils import run_bass_kernel_spmd

    x = np.asarray(x, dtype=np.float32)
    Wq = np.asarray(Wq, dtype=np.float32)
    Wk = np.asarray(Wk, dtype=np.float32)
    Wv = np.asarray(Wv, dtype=np.float32)
    Wo = np.asarray(Wo, dtype=np.float32)
    bo = np.asarray(bo, dtype=np.float32)

    nc = _get_nc()
    in_maps = _make_in_maps(x, Wq, Wk, Wv, Wo, bo)
    res = run_bass_kernel_spmd(nc, in_maps, list(range(N_CORES)))
    outs = [res.results[c]["out"] for c in range(N_CORES)]
    return np.stack([outs[2 * b] + outs[2 * b + 1] for b in range(B)], axis=0)


# revision 15
# speedup vs baseline: 1.5678x; 1.5678x over previous
"""Multi-head causal self-attention on 8 Trainium2 NeuronCores.

Sharding: 4-way data parallel over batch x 2-way tensor parallel over heads.
Core c handles batch c//2 and head group c%2 (8 of 16 heads). Partial
output projections (row-split Wo) are summed on the host; bias is added
on-device by the g=0 core (g=1 cores receive a zero bias).

v2 kernel: heads are processed in PAIRS. The score matmuls (contraction
dim = head_dim = 64) for the two heads of a pair are emitted back-to-back
with lhsT/rhs at base partitions 0 and 64, so the auto-derived
tile_position packs them into the top/bottom halves of the PE array and
they run concurrently (2x score throughput). Their outputs land in one
[128, 2048] PSUM region (head A cols 0:1024, head B 1024:2048) so a
single ACTIVATE computes exp for both heads (halves ACT call count).
P tiles are retained per 8-kt group and the ctx matmuls (V^T @ P, M=65
with a ones-column for softmax denominators) run chunk-outer with per-kt
narrowed column ranges (no zero-fill memsets); for the second q-half the
two kt-groups produce partial context sums combined in SBUF. Causal-diag
masks run on the otherwise-idle GPSIMD engine. QKV projection chains for
the second half and output-projection chunks are interleaved into the
attention emission as PE fill units so the tensor engine never idles
long enough for the HAM clock gate to re-throttle it to 1.2 GHz.
"""

import numpy as np

B, T, D = 4, 2048, 1024
HEADS = 16
N_CORES = 8
HPC = 8               # heads per core
HD = HPC * 64         # 512, per-core projection width
NT_D = D // 128       # 8 d-tiles
TB = 512              # t-block width for QKV streaming
NT_T = T // 128       # 16 t-tiles
HALF = 1024           # attention q-half width
CHUNK = 512
NPAIR = 4
KGRP = 8              # kt tiles per retained-P group

_NC = None


def _build():
    import concourse.tile as tile
    import concourse.mybir as mybir
    from concourse import bacc
    from contextlib import ExitStack
    from collections import deque

    F32 = mybir.dt.float32
    F32R = mybir.dt.float32r
    BF16 = mybir.dt.bfloat16
    EXP = mybir.ActivationFunctionType.Exp

    nc = bacc.Bacc("TRN2", target_bir_lowering=False, debug=False,
                   num_devices=N_CORES)

    xT_ext = nc.dram_tensor("xT", [D, T], BF16, kind="ExternalInput")
    wqT_ext = nc.dram_tensor("wqT", [D, HD], BF16, kind="ExternalInput")
    wkT_ext = nc.dram_tensor("wkT", [D, HD], BF16, kind="ExternalInput")
    wvT_ext = nc.dram_tensor("wvT", [D, HD], BF16, kind="ExternalInput")
    woT_ext = nc.dram_tensor("woT", [HD, D], BF16, kind="ExternalInput")
    bias_ext = nc.dram_tensor("bias", [1, D], F32, kind="ExternalInput")
    mask_ext = nc.dram_tensor("mask", [128, 128], BF16, kind="ExternalInput")
    out_ext = nc.dram_tensor("out", [T, D], F32, kind="ExternalOutput")
    import os as _os
    DBG = bool(_os.environ.get("KERNEL_DEBUG_TAPS"))
    if DBG:
        dbg_q = nc.dram_tensor("dbg_q", [128, T], BF16, kind="ExternalOutput")
        dbg_k = nc.dram_tensor("dbg_k", [128, T], BF16, kind="ExternalOutput")
        dbg_pt = nc.dram_tensor("dbg_pt", [128, 2 * HALF], BF16,
                                kind="ExternalOutput")
        dbg_lb = nc.dram_tensor("dbg_lb", [65, HALF], F32,
                                kind="ExternalOutput")
        dbg_ctx = nc.dram_tensor("dbg_ctx", [128, HALF], BF16,
                                 kind="ExternalOutput")

    with tile.TileContext(nc) as tc, ExitStack() as ctx:
        # ---- pools (PSUM: 4 + 2 + 2 = 8 banks) ------------------------
        wqkv_pool = ctx.enter_context(tc.tile_pool(name="wqkv", bufs=1))
        wo_pool = ctx.enter_context(tc.tile_pool(name="wo", bufs=1))
        qk_pool = ctx.enter_context(tc.tile_pool(name="qk", bufs=1))
        v_pool = ctx.enter_context(tc.tile_pool(name="v", bufs=1))
        xt_pool = ctx.enter_context(tc.tile_pool(name="xt", bufs=2))
        small = ctx.enter_context(tc.tile_pool(name="small", bufs=1))
        pt_pool = ctx.enter_context(tc.tile_pool(name="pt", bufs=1))
        ctxT_pool = ctx.enter_context(tc.tile_pool(name="ctxT", bufs=2))
        norm_pool = ctx.enter_context(tc.tile_pool(name="norm", bufs=2))
        lbuf_pool = ctx.enter_context(tc.tile_pool(name="lbuf", bufs=1))
        out_pool = ctx.enter_context(tc.tile_pool(name="outsb", bufs=3))
        s_ps_pool = ctx.enter_context(
            tc.tile_pool(name="sps", bufs=2, space="PSUM"))
        ctx_ps_pool = ctx.enter_context(
            tc.tile_pool(name="ctxps", bufs=2, space="PSUM"))
        mm_ps_pool = ctx.enter_context(
            tc.tile_pool(name="mmps", bufs=2, space="PSUM"))

        mask_sb = small.tile([128, 128], BF16, tag="mask")
        nc.sync.dma_start(mask_sb[:], mask_ext[:])
        bias_row = small.tile([1, D], F32, tag="biasrow")
        nc.sync.dma_start(bias_row[:], bias_ext[:])
        bias_bc = small.tile([128, D], F32, tag="biasbc")
        nc.gpsimd.partition_broadcast(bias_bc[:], bias_row[:])

        QT = [qk_pool.tile([128, T], BF16, tag=f"QT{i}", name=f"QT{i}")
              for i in range(NPAIR)]
        KT = [qk_pool.tile([128, T], BF16, tag=f"KT{i}", name=f"KT{i}")
              for i in range(NPAIR)]
        V = [v_pool.tile([128, HPC * 65], BF16, tag=f"V{i}", name=f"V{i}")
             for i in range(NT_T)]

        w_sb = {}

        def load_w(name, ext):
            t_ = wqkv_pool.tile([128, NT_D * HD], BF16, tag=f"w{name}",
                                name=f"w{name}")
            nc.sync.dma_start(
                t_[:].rearrange("p (dt c) -> p dt c", c=HD),
                ext[:].rearrange("(dt p) c -> p dt c", p=128))
            return [t_[:, dt * HD:(dt + 1) * HD] for dt in range(NT_D)]

        def load_x(tb):
            t_ = xt_pool.tile([128, NT_D * TB], BF16, tag=f"xt{tb % 2}",
                              name=f"xt{tb % 2}")
            nc.scalar.dma_start(
                t_[:].rearrange("p (dt c) -> p dt c", c=TB),
                xT_ext[:, tb * TB:(tb + 1) * TB]
                .rearrange("(dt p) c -> p dt c", p=128))
            return [t_[:, dt * TB:(dt + 1) * TB] for dt in range(NT_D)]

        def qk_chain(tb, xts, wname, dst, hdt):
            ps = mm_ps_pool.tile([128, TB], F32, tag="mm", name="mmps")
            for dt in range(NT_D):
                nc.tensor.matmul(
                    ps[:],
                    w_sb[wname][dt][:, hdt * 128:(hdt + 1) * 128],
                    xts[dt][:],
                    start=(dt == 0), stop=(dt == NT_D - 1))
            nc.vector.tensor_copy(
                dst[hdt][:, tb * TB:(tb + 1) * TB], ps[:])

        def v_chain(tb, xts, j):
            tt = tb * (TB // 128) + j
            ps = mm_ps_pool.tile([128, HD], F32, tag="mm", name="vps")
            for dt in range(NT_D):
                nc.tensor.matmul(
                    ps[:],
                    xts[dt][:, j * 128:(j + 1) * 128],
                    w_sb["v"][dt][:],
                    start=(dt == 0), stop=(dt == NT_D - 1))
            v3 = V[tt][:].rearrange("p (h c) -> p h c", c=65)
            nc.vector.memset(v3[:, :, 64:65], 1.0)
            nc.vector.tensor_copy(
                v3[:, :, 0:64],
                ps[:].rearrange("p (h c) -> p h c", c=64))

        def qkv_units(half):
            """Yield closures, each emitting one projection chain."""
            tbs = (0, 1) if half == 0 else (2, 3)
            if half == 0:
                w_sb["q"] = load_w("q", wqT_ext)
            xts_all = {tb: load_x(tb) for tb in tbs}
            if half == 0:
                w_sb["k"] = load_w("k", wkT_ext)
                w_sb["v"] = load_w("v", wvT_ext)
                wo_all = wo_pool.tile([128, NPAIR * D], BF16, tag="wo",
                                      name="wo")
                nc.sync.dma_start(
                    wo_all[:].rearrange("p (hdt c) -> p hdt c", c=D),
                    woT_ext[:].rearrange("(hdt p) c -> p hdt c", p=128))
                w_sb["o"] = [wo_all[:, hdt * D:(hdt + 1) * D]
                             for hdt in range(NPAIR)]
            q_units, kv_units = [], []
            for tb in tbs:
                for hdt in range(NPAIR):
                    q_units.append(
                        (lambda tb=tb, h=hdt:
                         qk_chain(tb, xts_all[tb], "q", QT, h)))
            for hdt in range(NPAIR):        # pair-major: KT[p] ready early
                for tb in tbs:
                    kv_units.append(
                        (lambda tb=tb, h=hdt:
                         qk_chain(tb, xts_all[tb], "k", KT, h)))
            for tb in tbs:
                for j in range(TB // 128):
                    kv_units.append(
                        (lambda tb=tb, j=j: v_chain(tb, xts_all[tb], j)))
            return deque(q_units), deque(kv_units)

        def scores_exp_pair(p, half, kt, pts):
            q0 = half * HALF
            off = max(0, kt * 128 - q0)
            pt = pt_pool.tile([128, 2 * HALF], BF16, tag=f"pt{kt % KGRP}",
                              name=f"pt{kt % KGRP}")
            pts[kt] = pt
            p3 = pt[:].rearrange("p (g q) -> p g q", q=HALF)
            for c in range(HALF // CHUNK):
                lo = c * CHUNK
                if lo + CHUNK <= off:
                    continue
                sub = max(0, off - lo)
                s = s_ps_pool.tile([128, 2 * CHUNK], F32, tag="s",
                                   name="sps")
                for h2 in range(2):
                    nc.tensor.matmul(
                        s[:, h2 * CHUNK + sub:(h2 + 1) * CHUNK],
                        KT[p][h2 * 64:h2 * 64 + 64,
                              kt * 128:(kt + 1) * 128],
                        QT[p][h2 * 64:h2 * 64 + 64,
                              q0 + lo + sub:q0 + lo + CHUNK],
                        start=True, stop=True)
                s3 = s[:].rearrange("p (g q) -> p g q", q=CHUNK)
                nc.scalar.activation(p3[:, :, lo + sub:lo + CHUNK],
                                     s3[:, :, sub:CHUNK], EXP, scale=0.125)
            if kt * 128 >= q0:
                for h2 in range(2):
                    nc.vector.tensor_mul(
                        pt[:, h2 * HALF + off:h2 * HALF + off + 128],
                        pt[:, h2 * HALF + off:h2 * HALF + off + 128],
                        mask_sb[:])

        def ctx_unit(p, half, h2, c, pts, ctxT_h, lbuf, first):
            """Context-chain over the kt's present in pts for chunk c."""
            q0 = half * HALF
            h = 2 * p + h2
            lo = c * CHUNK
            kts = [kt for kt in sorted(pts)
                   if max(0, kt * 128 - q0) < lo + CHUNK]
            if not kts:
                return
            cps = ctx_ps_pool.tile([65, CHUNK], F32, tag="ctx", name="ctxps")
            for i, kt in enumerate(kts):
                sub = max(0, max(0, kt * 128 - q0) - lo)
                nc.tensor.matmul(
                    cps[:, sub:CHUNK],
                    V[kt][:, h * 65:(h + 1) * 65],
                    pts[kt][:, h2 * HALF + lo + sub:h2 * HALF + lo + CHUNK],
                    start=(i == 0), stop=(i == len(kts) - 1),
                    skip_group_check=True)
            dst = ctxT_h[p][h2 * 64:h2 * 64 + 64, lo:lo + CHUNK]
            dden = lbuf[p][h2 * 64:h2 * 64 + 1, lo:lo + CHUNK]
            if first:
                nc.vector.tensor_copy(dst, cps[0:64, :])
                nc.vector.tensor_copy(dden, cps[64:65, :])
            else:
                nc.vector.tensor_add(dst, dst, cps[0:64, :])
                nc.vector.tensor_add(dden, dden, cps[64:65, :])

        def norm_pair(ctxT_h, lbuf, p):
            for cc in range(HALF // CHUNK):
                sl = slice(cc * CHUNK, (cc + 1) * CHUNK)
                nc.vector.reciprocal(lbuf[p][:, sl], lbuf[p][:, sl])
                for h2 in range(2):
                    po = h2 * 64
                    stage = norm_pool.tile([1, CHUNK], BF16, tag="stage",
                                           name="stage")
                    nc.vector.tensor_copy(stage[:],
                                          lbuf[p][po:po + 1, sl])
                    bc = norm_pool.tile([128, CHUNK], BF16, tag="bc",
                                        name="bc")
                    nc.gpsimd.partition_broadcast(bc[:], stage[:])
                    nc.vector.tensor_mul(
                        ctxT_h[p][po:po + 64, sl],
                        ctxT_h[p][po:po + 64, sl],
                        bc[po:po + 64, :])

        def outproj_units(half, ctxT_h):
            q0 = half * HALF
            units = []

            def one(tt, ob):
                ps = mm_ps_pool.tile([128, CHUNK], F32, tag="mm", name="opps")
                for p in range(NPAIR):
                    nc.tensor.matmul(
                        ps[:],
                        ctxT_h[p][:, tt * 128:(tt + 1) * 128],
                        w_sb["o"][p][:, ob * CHUNK:(ob + 1) * CHUNK],
                        start=(p == 0), stop=(p == NPAIR - 1))
                osb = out_pool.tile([128, CHUNK], F32, tag="osb", name="osb")
                nc.vector.tensor_add(
                    osb[:], ps[:], bias_bc[:, ob * CHUNK:(ob + 1) * CHUNK])
                nc.sync.dma_start(
                    out_ext[q0 + tt * 128:q0 + (tt + 1) * 128,
                            ob * CHUNK:(ob + 1) * CHUNK],
                    osb[:])

            for tt in range(HALF // 128):
                for ob in range(D // CHUNK):
                    units.append(lambda tt=tt, ob=ob: one(tt, ob))
            return deque(units)

        def attn_half(half, fill):
            q0 = half * HALF
            nkt = (q0 + HALF) // 128
            ctxT_h = [ctxT_pool.tile([128, HALF], BF16, tag=f"ctxT{i}",
                                     name=f"ctxT{i}")
                      for i in range(NPAIR)]
            lbuf = [lbuf_pool.tile([65, HALF], F32, tag=f"lb{j}",
                                   name=f"lb{j}")
                    for j in range(NPAIR)]
            for p in range(NPAIR):
                for g in range(nkt // KGRP):
                    pts = {}
                    for kt in range(g * KGRP, (g + 1) * KGRP):
                        scores_exp_pair(p, half, kt, pts)
                        if DBG and half == 0 and p == 0 and kt == 0:
                            nc.sync.dma_start(dbg_pt[:], pts[0][:])
                        if fill:
                            fill.popleft()()
                    for c in range(HALF // CHUNK):
                        for h2 in range(2):
                            ctx_unit(p, half, h2, c, pts, ctxT_h, lbuf,
                                     first=(g == 0))
                norm_pair(ctxT_h, lbuf, p)
            if DBG and half == 0:
                nc.sync.dma_start(dbg_lb[:], lbuf[0][:])
            if DBG and half == 0:
                nc.sync.dma_start(dbg_ctx[:], ctxT_h[0][:])
                nc.sync.dma_start(dbg_q[:], QT[0][:])
                nc.sync.dma_start(dbg_k[:], KT[0][:])
            return ctxT_h

        # ---------------- emission schedule ----------------------------
        q0u, kv0u = qkv_units(0)
        while q0u:
            q0u.popleft()()
        while kv0u:
            kv0u.popleft()()

        q1u, kv1u = qkv_units(1)        # emits x(h1) load DMAs up front
        ctxT_0 = attn_half(0, q1u)      # Q(h1) chains fill attn(h0)
        while q1u:
            q1u.popleft()()

        fill1 = kv1u                    # K/V(h1) chains fill attn(h1) grp 0
        ctxT_1_box = []

        def run_attn1():
            # outproj(h0) chunks join the fill queue once norm(h0) is done
            fill1.extend(outproj_units(0, ctxT_0))
            ctxT_1_box.append(attn_half(1, fill1))

        run_attn1()
        while fill1:
            fill1.popleft()()

        op1 = outproj_units(1, ctxT_1_box[0])
        while op1:
            op1.popleft()()

    nc.compile()
    return nc


def _get_nc():
    global _NC
    if _NC is None:
        _NC = _build()
    return _NC


def _make_in_maps(x, Wq, Wk, Wv, Wo, bo):
    import ml_dtypes
    mask = np.triu(np.ones((128, 128), dtype=np.float32)).astype(
        ml_dtypes.bfloat16)
    zero_bias = np.zeros((1, D), dtype=np.float32)
    xT = [np.ascontiguousarray(x[b].T) for b in range(B)]
    in_maps = []
    for c in range(N_CORES):
        b, g = c // 2, c % 2
        sl = slice(g * HD, (g + 1) * HD)
        in_maps.append({
            "xT": xT[b].astype(ml_dtypes.bfloat16),
            "wqT": np.ascontiguousarray(Wq[sl, :].T).astype(ml_dtypes.bfloat16),
            "wkT": np.ascontiguousarray(Wk[sl, :].T).astype(ml_dtypes.bfloat16),
            "wvT": np.ascontiguousarray(Wv[sl, :].T).astype(ml_dtypes.bfloat16),
            "woT": np.ascontiguousarray(Wo[:, sl].T).astype(ml_dtypes.bfloat16),
            "bias": bo.reshape(1, D) if g == 0 else zero_bias,
            "mask": mask,
        })
    return in_maps


def kernel(x, Wq, Wk, Wv, Wo, bo):
    from concourse.bass_utils import run_bass_kernel_spmd

    x = np.asarray(x, dtype=np.float32)
    Wq = np.asarray(Wq, dtype=np.float32)
    Wk = np.asarray(Wk, dtype=np.float32)
    Wv = np.asarray(Wv, dtype=np.float32)
    Wo = np.asarray(Wo, dtype=np.float32)
    bo = np.asarray(bo, dtype=np.float32)

    nc = _get_nc()
    in_maps = _make_in_maps(x, Wq, Wk, Wv, Wo, bo)
    res = run_bass_kernel_spmd(nc, in_maps, list(range(N_CORES)))
    outs = [res.results[c]["out"] for c in range(N_CORES)]
    return np.stack([outs[2 * b] + outs[2 * b + 1] for b in range(B)], axis=0)


# revision 17
# speedup vs baseline: 1.5972x; 1.0188x over previous
"""Multi-head causal self-attention on 8 Trainium2 NeuronCores.

Sharding: 4-way data parallel over batch x 2-way tensor parallel over heads.
Core c handles batch c//2 and head group c%2 (8 of 16 heads). Partial
output projections (row-split Wo) are summed on the host; bias is added
on-device by the g=0 core (g=1 cores receive a zero bias).

v2 kernel: heads are processed in PAIRS. The score matmuls (contraction
dim = head_dim = 64) for the two heads of a pair are emitted back-to-back
with lhsT/rhs at base partitions 0 and 64, so the auto-derived
tile_position packs them into the top/bottom halves of the PE array and
they run concurrently (2x score throughput). Their outputs land in one
[128, 2048] PSUM region (head A cols 0:1024, head B 1024:2048) so a
single ACTIVATE computes exp for both heads (halves ACT call count).
P tiles are retained per 8-kt group and the ctx matmuls (V^T @ P, M=65
with a ones-column for softmax denominators) run chunk-outer with per-kt
narrowed column ranges (no zero-fill memsets); for the second q-half the
two kt-groups produce partial context sums combined in SBUF. Causal-diag
masks run on the otherwise-idle GPSIMD engine. QKV projection chains for
the second half and output-projection chunks are interleaved into the
attention emission as PE fill units so the tensor engine never idles
long enough for the HAM clock gate to re-throttle it to 1.2 GHz.
"""

import numpy as np

B, T, D = 4, 2048, 1024
HEADS = 16
N_CORES = 8
HPC = 8               # heads per core
HD = HPC * 64         # 512, per-core projection width
NT_D = D // 128       # 8 d-tiles
TB = 512              # t-block width for QKV streaming
NT_T = T // 128       # 16 t-tiles
HALF = 1024           # attention q-half width
CHUNK = 512
NPAIR = 4
KGRP = 8              # kt tiles per retained-P group

_NC = None


def _build():
    import concourse.tile as tile
    import concourse.mybir as mybir
    from concourse import bacc
    from contextlib import ExitStack
    from collections import deque

    F32 = mybir.dt.float32
    F32R = mybir.dt.float32r
    BF16 = mybir.dt.bfloat16
    EXP = mybir.ActivationFunctionType.Exp

    nc = bacc.Bacc("TRN2", target_bir_lowering=False, debug=False,
                   num_devices=N_CORES)

    xT_ext = nc.dram_tensor("xT", [D, T], BF16, kind="ExternalInput")
    wqT_ext = nc.dram_tensor("wqT", [D, HD], BF16, kind="ExternalInput")
    wkT_ext = nc.dram_tensor("wkT", [D, HD], BF16, kind="ExternalInput")
    wvT_ext = nc.dram_tensor("wvT", [D, HD], BF16, kind="ExternalInput")
    woT_ext = nc.dram_tensor("woT", [HD, D], BF16, kind="ExternalInput")
    bias_ext = nc.dram_tensor("bias", [1, D], F32, kind="ExternalInput")
    mask_ext = nc.dram_tensor("mask", [128, 128], BF16, kind="ExternalInput")
    out_ext = nc.dram_tensor("out", [T, D], F32, kind="ExternalOutput")
    import os as _os
    DBG = bool(_os.environ.get("KERNEL_DEBUG_TAPS"))
    if DBG:
        dbg_q = nc.dram_tensor("dbg_q", [128, T], BF16, kind="ExternalOutput")
        dbg_k = nc.dram_tensor("dbg_k", [128, T], BF16, kind="ExternalOutput")
        dbg_pt = nc.dram_tensor("dbg_pt", [128, 2 * HALF], BF16,
                                kind="ExternalOutput")
        dbg_lb = nc.dram_tensor("dbg_lb", [65, HALF], F32,
                                kind="ExternalOutput")
        dbg_ctx = nc.dram_tensor("dbg_ctx", [128, HALF], BF16,
                                 kind="ExternalOutput")

    with tile.TileContext(nc) as tc, ExitStack() as ctx:
        # ---- pools (PSUM: 4 + 2 + 2 = 8 banks) ------------------------
        wqkv_pool = ctx.enter_context(tc.tile_pool(name="wqkv", bufs=1))
        wo_pool = ctx.enter_context(tc.tile_pool(name="wo", bufs=1))
        qk_pool = ctx.enter_context(tc.tile_pool(name="qk", bufs=1))
        v_pool = ctx.enter_context(tc.tile_pool(name="v", bufs=1))
        xt_pool = ctx.enter_context(tc.tile_pool(name="xt", bufs=2))
        small = ctx.enter_context(tc.tile_pool(name="small", bufs=1))
        pt_pool = ctx.enter_context(tc.tile_pool(name="pt", bufs=1))
        ctxT_pool = ctx.enter_context(tc.tile_pool(name="ctxT", bufs=2))
        norm_pool = ctx.enter_context(tc.tile_pool(name="norm", bufs=2))
        lbuf_pool = ctx.enter_context(tc.tile_pool(name="lbuf", bufs=1))
        out_pool = ctx.enter_context(tc.tile_pool(name="outsb", bufs=3))
        s_ps_pool = ctx.enter_context(
            tc.tile_pool(name="sps", bufs=2, space="PSUM"))
        ctx_ps_pool = ctx.enter_context(
            tc.tile_pool(name="ctxps", bufs=2, space="PSUM"))
        mm_ps_pool = ctx.enter_context(
            tc.tile_pool(name="mmps", bufs=2, space="PSUM"))

        mask_sb = small.tile([128, 128], BF16, tag="mask")
        nc.sync.dma_start(mask_sb[:], mask_ext[:])
        bias_row = small.tile([1, D], F32, tag="biasrow")
        nc.sync.dma_start(bias_row[:], bias_ext[:])
        bias_bc = small.tile([128, D], F32, tag="biasbc")
        nc.gpsimd.partition_broadcast(bias_bc[:], bias_row[:])

        QT = [qk_pool.tile([128, T], BF16, tag=f"QT{i}", name=f"QT{i}")
              for i in range(NPAIR)]
        KT = [qk_pool.tile([128, T], BF16, tag=f"KT{i}", name=f"KT{i}")
              for i in range(NPAIR)]
        V = [v_pool.tile([128, HPC * 65], BF16, tag=f"V{i}", name=f"V{i}")
             for i in range(NT_T)]

        w_sb = {}

        def load_w(name, ext):
            t_ = wqkv_pool.tile([128, NT_D * HD], BF16, tag=f"w{name}",
                                name=f"w{name}")
            nc.sync.dma_start(
                t_[:].rearrange("p (dt c) -> p dt c", c=HD),
                ext[:].rearrange("(dt p) c -> p dt c", p=128))
            return [t_[:, dt * HD:(dt + 1) * HD] for dt in range(NT_D)]

        def load_x(tb):
            t_ = xt_pool.tile([128, NT_D * TB], BF16, tag=f"xt{tb % 2}",
                              name=f"xt{tb % 2}")
            nc.sync.dma_start(
                t_[:].rearrange("p (dt c) -> p dt c", c=TB),
                xT_ext[:, tb * TB:(tb + 1) * TB]
                .rearrange("(dt p) c -> p dt c", p=128))
            return [t_[:, dt * TB:(dt + 1) * TB] for dt in range(NT_D)]

        def qk_chain(tb, xts, wname, dst, hdt):
            ps = mm_ps_pool.tile([128, TB], F32, tag="mm", name="mmps")
            for dt in range(NT_D):
                nc.tensor.matmul(
                    ps[:],
                    w_sb[wname][dt][:, hdt * 128:(hdt + 1) * 128],
                    xts[dt][:],
                    start=(dt == 0), stop=(dt == NT_D - 1))
            nc.vector.tensor_copy(
                dst[hdt][:, tb * TB:(tb + 1) * TB], ps[:])

        def v_chain(tb, xts, j):
            tt = tb * (TB // 128) + j
            ps = mm_ps_pool.tile([128, HD], F32, tag="mm", name="vps")
            for dt in range(NT_D):
                nc.tensor.matmul(
                    ps[:],
                    xts[dt][:, j * 128:(j + 1) * 128],
                    w_sb["v"][dt][:],
                    start=(dt == 0), stop=(dt == NT_D - 1))
            v3 = V[tt][:].rearrange("p (h c) -> p h c", c=65)
            nc.vector.memset(v3[:, :, 64:65], 1.0)
            nc.vector.tensor_copy(
                v3[:, :, 0:64],
                ps[:].rearrange("p (h c) -> p h c", c=64))

        def qkv_units(half):
            """Yield closures, each emitting one projection chain."""
            tbs = (0, 1) if half == 0 else (2, 3)
            if half == 0:
                w_sb["q"] = load_w("q", wqT_ext)
            xts_all = {tb: load_x(tb) for tb in tbs}
            if half == 0:
                w_sb["k"] = load_w("k", wkT_ext)
                w_sb["v"] = load_w("v", wvT_ext)
                wo_all = wo_pool.tile([128, NPAIR * D], BF16, tag="wo",
                                      name="wo")
                nc.sync.dma_start(
                    wo_all[:].rearrange("p (hdt c) -> p hdt c", c=D),
                    woT_ext[:].rearrange("(hdt p) c -> p hdt c", p=128))
                w_sb["o"] = [wo_all[:, hdt * D:(hdt + 1) * D]
                             for hdt in range(NPAIR)]
            q_units, kv_units = [], []
            for tb in tbs:
                for hdt in range(NPAIR):
                    q_units.append(
                        (lambda tb=tb, h=hdt:
                         qk_chain(tb, xts_all[tb], "q", QT, h)))
            for hdt in range(NPAIR):        # pair-major: KT[p] ready early
                for tb in tbs:
                    kv_units.append(
                        (lambda tb=tb, h=hdt:
                         qk_chain(tb, xts_all[tb], "k", KT, h)))
            for tb in tbs:
                for j in range(TB // 128):
                    kv_units.append(
                        (lambda tb=tb, j=j: v_chain(tb, xts_all[tb], j)))
            return deque(q_units), deque(kv_units)

        def scores_exp_pair(p, half, kt, pts):
            q0 = half * HALF
            off = max(0, kt * 128 - q0)
            pt = pt_pool.tile([128, 2 * HALF], BF16, tag=f"pt{kt % KGRP}",
                              name=f"pt{kt % KGRP}")
            pts[kt] = pt
            p3 = pt[:].rearrange("p (g q) -> p g q", q=HALF)
            for c in range(HALF // CHUNK):
                lo = c * CHUNK
                if lo + CHUNK <= off:
                    continue
                sub = max(0, off - lo)
                s = s_ps_pool.tile([128, 2 * CHUNK], F32, tag="s",
                                   name="sps")
                for h2 in range(2):
                    nc.tensor.matmul(
                        s[:, h2 * CHUNK + sub:(h2 + 1) * CHUNK],
                        KT[p][h2 * 64:h2 * 64 + 64,
                              kt * 128:(kt + 1) * 128],
                        QT[p][h2 * 64:h2 * 64 + 64,
                              q0 + lo + sub:q0 + lo + CHUNK],
                        start=True, stop=True)
                s3 = s[:].rearrange("p (g q) -> p g q", q=CHUNK)
                nc.scalar.activation(p3[:, :, lo + sub:lo + CHUNK],
                                     s3[:, :, sub:CHUNK], EXP, scale=0.125)
            if kt * 128 >= q0:
                nc.vector.tensor_mul(
                    p3[:, :, off:off + 128],
                    p3[:, :, off:off + 128],
                    mask_sb[:].rearrange("p (o q) -> p o q", o=1)
                    .broadcast_to([128, 2, 128]))

        def ctx_unit(p, half, h2, c, pts, ctxT_h, lbuf, first):
            """Context-chain over the kt's present in pts for chunk c."""
            q0 = half * HALF
            h = 2 * p + h2
            lo = c * CHUNK
            kts = [kt for kt in sorted(pts)
                   if max(0, kt * 128 - q0) < lo + CHUNK]
            if not kts:
                return
            cps = ctx_ps_pool.tile([65, CHUNK], F32, tag="ctx", name="ctxps")
            for i, kt in enumerate(kts):
                sub = max(0, max(0, kt * 128 - q0) - lo)
                nc.tensor.matmul(
                    cps[:, sub:CHUNK],
                    V[kt][:, h * 65:(h + 1) * 65],
                    pts[kt][:, h2 * HALF + lo + sub:h2 * HALF + lo + CHUNK],
                    start=(i == 0), stop=(i == len(kts) - 1),
                    skip_group_check=True)
            dst = ctxT_h[p][h2 * 64:h2 * 64 + 64, lo:lo + CHUNK]
            dden = lbuf[p][h2 * 64:h2 * 64 + 1, lo:lo + CHUNK]
            if first:
                nc.vector.tensor_copy(dst, cps[0:64, :])
                nc.vector.tensor_copy(dden, cps[64:65, :])
            else:
                nc.vector.tensor_add(dst, dst, cps[0:64, :])
                nc.vector.tensor_add(dden, dden, cps[64:65, :])

        def norm_pair_cc(ctxT_h, lbuf, p, cc):
            sl = slice(cc * CHUNK, (cc + 1) * CHUNK)
            nc.vector.reciprocal(lbuf[p][:, sl], lbuf[p][:, sl])
            for h2 in range(2):
                po = h2 * 64
                stage = norm_pool.tile([1, CHUNK], BF16, tag="stage",
                                       name="stage")
                nc.vector.tensor_copy(stage[:], lbuf[p][po:po + 1, sl])
                bc = norm_pool.tile([128, CHUNK], BF16, tag="bc",
                                    name="bc")
                nc.gpsimd.partition_broadcast(bc[:], stage[:])
                nc.vector.tensor_mul(
                    ctxT_h[p][po:po + 64, sl],
                    ctxT_h[p][po:po + 64, sl],
                    bc[po:po + 64, :])

        def outproj_units(half, ctxT_h):
            q0 = half * HALF
            units = []

            def one(tt, ob):
                ps = mm_ps_pool.tile([128, CHUNK], F32, tag="mm", name="opps")
                for p in range(NPAIR):
                    nc.tensor.matmul(
                        ps[:],
                        ctxT_h[p][:, tt * 128:(tt + 1) * 128],
                        w_sb["o"][p][:, ob * CHUNK:(ob + 1) * CHUNK],
                        start=(p == 0), stop=(p == NPAIR - 1))
                osb = out_pool.tile([128, CHUNK], F32, tag="osb", name="osb")
                nc.vector.tensor_add(
                    osb[:], ps[:], bias_bc[:, ob * CHUNK:(ob + 1) * CHUNK])
                nc.sync.dma_start(
                    out_ext[q0 + tt * 128:q0 + (tt + 1) * 128,
                            ob * CHUNK:(ob + 1) * CHUNK],
                    osb[:])

            for tt in range(HALF // 128):
                for ob in range(D // CHUNK):
                    units.append(lambda tt=tt, ob=ob: one(tt, ob))
            return deque(units)

        def attn_half(half, fill):
            aux = deque()
            q0 = half * HALF
            nkt = (q0 + HALF) // 128
            ctxT_h = [ctxT_pool.tile([128, HALF], BF16, tag=f"ctxT{i}",
                                     name=f"ctxT{i}")
                      for i in range(NPAIR)]
            lbuf = [lbuf_pool.tile([65, HALF], F32, tag=f"lb{j}",
                                   name=f"lb{j}")
                    for j in range(NPAIR)]
            for p in range(NPAIR):
                for g in range(nkt // KGRP):
                    pts = {}
                    for kt in range(g * KGRP, (g + 1) * KGRP):
                        scores_exp_pair(p, half, kt, pts)
                        if DBG and half == 0 and p == 0 and kt == 0:
                            nc.sync.dma_start(dbg_pt[:], pts[0][:])
                        if aux:
                            aux.popleft()()
                        if fill:
                            fill.popleft()()
                    for c in range(HALF // CHUNK):
                        for h2 in range(2):
                            ctx_unit(p, half, h2, c, pts, ctxT_h, lbuf,
                                     first=(g == 0))
                for cc in range(HALF // CHUNK):
                    aux.append(lambda p=p, cc=cc:
                               norm_pair_cc(ctxT_h, lbuf, p, cc))
            while aux:
                aux.popleft()()
            if DBG and half == 0:
                nc.sync.dma_start(dbg_lb[:], lbuf[0][:])
            if DBG and half == 0:
                nc.sync.dma_start(dbg_ctx[:], ctxT_h[0][:])
                nc.sync.dma_start(dbg_q[:], QT[0][:])
                nc.sync.dma_start(dbg_k[:], KT[0][:])
            return ctxT_h

        # ---------------- emission schedule ----------------------------
        q0u, kv0u = qkv_units(0)
        while q0u:
            q0u.popleft()()
        while kv0u:
            kv0u.popleft()()

        q1u, kv1u = qkv_units(1)        # emits x(h1) load DMAs up front
        ctxT_0 = attn_half(0, q1u)      # Q(h1) chains fill attn(h0)
        while q1u:
            q1u.popleft()()

        fill1 = kv1u                    # K/V(h1) chains fill attn(h1) grp 0
        ctxT_1_box = []

        def run_attn1():
            # outproj(h0) chunks join the fill queue once norm(h0) is done
            fill1.extend(outproj_units(0, ctxT_0))
            ctxT_1_box.append(attn_half(1, fill1))

        run_attn1()
        while fill1:
            fill1.popleft()()

        op1 = outproj_units(1, ctxT_1_box[0])
        while op1:
            op1.popleft()()

    nc.compile()
    return nc


def _get_nc():
    global _NC
    if _NC is None:
        _NC = _build()
    return _NC


def _make_in_maps(x, Wq, Wk, Wv, Wo, bo):
    import ml_dtypes
    mask = np.triu(np.ones((128, 128), dtype=np.float32)).astype(
        ml_dtypes.bfloat16)
    zero_bias = np.zeros((1, D), dtype=np.float32)
    xT = [np.ascontiguousarray(x[b].T) for b in range(B)]
    in_maps = []
    for c in range(N_CORES):
        b, g = c // 2, c % 2
        sl = slice(g * HD, (g + 1) * HD)
        in_maps.append({
            "xT": xT[b].astype(ml_dtypes.bfloat16),
            "wqT": np.ascontiguousarray(Wq[sl, :].T).astype(ml_dtypes.bfloat16),
            "wkT": np.ascontiguousarray(Wk[sl, :].T).astype(ml_dtypes.bfloat16),
            "wvT": np.ascontiguousarray(Wv[sl, :].T).astype(ml_dtypes.bfloat16),
            "woT": np.ascontiguousarray(Wo[:, sl].T).astype(ml_dtypes.bfloat16),
            "bias": bo.reshape(1, D) if g == 0 else zero_bias,
            "mask": mask,
        })
    return in_maps


def kernel(x, Wq, Wk, Wv, Wo, bo):
    from concourse.bass_utils import run_bass_kernel_spmd

    x = np.asarray(x, dtype=np.float32)
    Wq = np.asarray(Wq, dtype=np.float32)
    Wk = np.asarray(Wk, dtype=np.float32)
    Wv = np.asarray(Wv, dtype=np.float32)
    Wo = np.asarray(Wo, dtype=np.float32)
    bo = np.asarray(bo, dtype=np.float32)

    nc = _get_nc()
    in_maps = _make_in_maps(x, Wq, Wk, Wv, Wo, bo)
    res = run_bass_kernel_spmd(nc, in_maps, list(range(N_CORES)))
    outs = [res.results[c]["out"] for c in range(N_CORES)]
    return np.stack([outs[2 * b] + outs[2 * b + 1] for b in range(B)], axis=0)


# revision 19
# speedup vs baseline: 1.8109x; 1.1338x over previous
"""Multi-head causal self-attention on 8 Trainium2 NeuronCores.

Sharding: 4-way data parallel over batch x 2-way tensor parallel over heads.
Core c handles batch c//2 and head group c%2 (8 of 16 heads). Partial
output projections (row-split Wo) are summed on the host; bias is added
on-device by the g=0 core (g=1 cores receive a zero bias).

v2 kernel: heads are processed in PAIRS. The score matmuls (contraction
dim = head_dim = 64) for the two heads of a pair are emitted back-to-back
with lhsT/rhs at base partitions 0 and 64, so the auto-derived
tile_position packs them into the top/bottom halves of the PE array and
they run concurrently (2x score throughput). Their outputs land in one
[128, 2048] PSUM region (head A cols 0:1024, head B 1024:2048) so a
single ACTIVATE computes exp for both heads (halves ACT call count).
P tiles are retained per 8-kt group and the ctx matmuls (V^T @ P, M=65
with a ones-column for softmax denominators) run chunk-outer with per-kt
narrowed column ranges (no zero-fill memsets); for the second q-half the
two kt-groups produce partial context sums combined in SBUF. Causal-diag
masks run on the otherwise-idle GPSIMD engine. QKV projection chains for
the second half and output-projection chunks are interleaved into the
attention emission as PE fill units so the tensor engine never idles
long enough for the HAM clock gate to re-throttle it to 1.2 GHz.
"""

import numpy as np

B, T, D = 4, 2048, 1024
HEADS = 16
N_CORES = 8
HPC = 8               # heads per core
HD = HPC * 64         # 512, per-core projection width
NT_D = D // 128       # 8 d-tiles
TB = 512              # t-block width for QKV streaming
NT_T = T // 128       # 16 t-tiles
HALF = 1024           # attention q-half width
CHUNK = 512
NPAIR = 4
KGRP = 8              # kt tiles per retained-P group

_NC = None


def _build():
    import concourse.tile as tile
    import concourse.mybir as mybir
    from concourse import bacc
    from contextlib import ExitStack
    from collections import deque

    F32 = mybir.dt.float32
    F32R = mybir.dt.float32r
    BF16 = mybir.dt.bfloat16
    EXP = mybir.ActivationFunctionType.Exp

    nc = bacc.Bacc("TRN2", target_bir_lowering=False, debug=False,
                   num_devices=N_CORES)

    xT_ext = nc.dram_tensor("xT", [D, T], BF16, kind="ExternalInput")
    wqT_ext = nc.dram_tensor("wqT", [D, HD], BF16, kind="ExternalInput")
    wkT_ext = nc.dram_tensor("wkT", [D, HD], BF16, kind="ExternalInput")
    wvT_ext = nc.dram_tensor("wvT", [D, HD], BF16, kind="ExternalInput")
    woT_ext = nc.dram_tensor("woT", [HD, D], BF16, kind="ExternalInput")
    bias_ext = nc.dram_tensor("bias", [1, D], F32, kind="ExternalInput")
    mask_ext = nc.dram_tensor("mask", [128, 128], BF16, kind="ExternalInput")
    out_ext = nc.dram_tensor("out", [T, D], F32, kind="ExternalOutput")
    import os as _os
    DBG = bool(_os.environ.get("KERNEL_DEBUG_TAPS"))
    if DBG:
        dbg_q = nc.dram_tensor("dbg_q", [128, T], BF16, kind="ExternalOutput")
        dbg_k = nc.dram_tensor("dbg_k", [128, T], BF16, kind="ExternalOutput")
        dbg_pt = nc.dram_tensor("dbg_pt", [128, 2 * HALF], BF16,
                                kind="ExternalOutput")
        dbg_lb = nc.dram_tensor("dbg_lb", [65, HALF], F32,
                                kind="ExternalOutput")
        dbg_ctx = nc.dram_tensor("dbg_ctx", [128, HALF], BF16,
                                 kind="ExternalOutput")

    with tile.TileContext(nc) as tc, ExitStack() as ctx:
        # ---- pools (PSUM: 4 + 2 + 2 = 8 banks) ------------------------
        wqkv_pool = ctx.enter_context(tc.tile_pool(name="wqkv", bufs=1))
        wo_pool = ctx.enter_context(tc.tile_pool(name="wo", bufs=1))
        qk_pool = ctx.enter_context(tc.tile_pool(name="qk", bufs=1))
        v_pool = ctx.enter_context(tc.tile_pool(name="v", bufs=1))
        xt_pool = ctx.enter_context(tc.tile_pool(name="xt", bufs=2))
        small = ctx.enter_context(tc.tile_pool(name="small", bufs=1))
        pt_pool = ctx.enter_context(tc.tile_pool(name="pt", bufs=1))
        ctxT_pool = ctx.enter_context(tc.tile_pool(name="ctxT", bufs=2))
        norm_pool = ctx.enter_context(tc.tile_pool(name="norm", bufs=2))
        lbuf_pool = ctx.enter_context(tc.tile_pool(name="lbuf", bufs=1))
        out_pool = ctx.enter_context(tc.tile_pool(name="outsb", bufs=3))
        s_ps_pool = ctx.enter_context(
            tc.tile_pool(name="sps", bufs=2, space="PSUM"))
        ctx_ps_pool = ctx.enter_context(
            tc.tile_pool(name="ctxps", bufs=2, space="PSUM"))
        mm_ps_pool = ctx.enter_context(
            tc.tile_pool(name="mmps", bufs=2, space="PSUM"))

        mask_sb = small.tile([128, 128], BF16, tag="mask")
        nc.gpsimd.dma_start(mask_sb[:], mask_ext[:])
        bias_row = small.tile([1, D], F32, tag="biasrow")
        nc.gpsimd.dma_start(bias_row[:], bias_ext[:])
        bias_bc = small.tile([128, D], F32, tag="biasbc")
        nc.gpsimd.partition_broadcast(bias_bc[:], bias_row[:])

        QT = [qk_pool.tile([128, T], BF16, tag=f"QT{i}", name=f"QT{i}")
              for i in range(NPAIR)]
        KT = [qk_pool.tile([128, T], BF16, tag=f"KT{i}", name=f"KT{i}")
              for i in range(NPAIR)]
        V = [v_pool.tile([128, HPC * 65], BF16, tag=f"V{i}", name=f"V{i}")
             for i in range(NT_T)]

        w_sb = {}

        def load_w(name, ext):
            t_ = wqkv_pool.tile([128, NT_D * HD], BF16, tag=f"w{name}",
                                name=f"w{name}")
            nc.gpsimd.dma_start(
                t_[:].rearrange("p (dt c) -> p dt c", c=HD),
                ext[:].rearrange("(dt p) c -> p dt c", p=128))
            return [t_[:, dt * HD:(dt + 1) * HD] for dt in range(NT_D)]

        def load_x(tb):
            t_ = xt_pool.tile([128, NT_D * TB], BF16, tag=f"xt{tb % 2}",
                              name=f"xt{tb % 2}")
            hw = NT_D // 2 * TB
            for i in range(2):
                nc.sync.dma_start(
                    t_[:, i * hw:(i + 1) * hw]
                    .rearrange("p (dt c) -> p dt c", c=TB),
                    xT_ext[i * (D // 2):(i + 1) * (D // 2),
                           tb * TB:(tb + 1) * TB]
                    .rearrange("(dt p) c -> p dt c", p=128))
            return [t_[:, dt * TB:(dt + 1) * TB] for dt in range(NT_D)]

        def qk_chain(tb, xts, wname, dst, hdt, on_act=False):
            ps = mm_ps_pool.tile([128, TB], F32, tag="mm", name="mmps")
            for dt in range(NT_D):
                nc.tensor.matmul(
                    ps[:],
                    w_sb[wname][dt][:, hdt * 128:(hdt + 1) * 128],
                    xts[dt][:],
                    start=(dt == 0), stop=(dt == NT_D - 1))
            if on_act:
                nc.scalar.copy(dst[hdt][:, tb * TB:(tb + 1) * TB], ps[:])
            else:
                nc.vector.tensor_copy(
                    dst[hdt][:, tb * TB:(tb + 1) * TB], ps[:])

        def v_chain(tb, xts, j, on_act=False):
            tt = tb * (TB // 128) + j
            ps = mm_ps_pool.tile([128, HD], F32, tag="mm", name="vps")
            for dt in range(NT_D):
                nc.tensor.matmul(
                    ps[:],
                    xts[dt][:, j * 128:(j + 1) * 128],
                    w_sb["v"][dt][:],
                    start=(dt == 0), stop=(dt == NT_D - 1))
            v3 = V[tt][:].rearrange("p (h c) -> p h c", c=65)
            nc.vector.memset(v3[:, :, 64:65], 1.0)
            if on_act:
                nc.scalar.copy(v3[:, :, 0:64],
                               ps[:].rearrange("p (h c) -> p h c", c=64))
            else:
                nc.vector.tensor_copy(
                    v3[:, :, 0:64],
                    ps[:].rearrange("p (h c) -> p h c", c=64))

        def qkv_units(half):
            """Yield closures, each emitting one projection chain."""
            tbs = (0, 1) if half == 0 else (2, 3)
            if half == 0:
                w_sb["q"] = load_w("q", wqT_ext)
            xts_all = {tb: load_x(tb) for tb in tbs}
            if half == 0:
                w_sb["k"] = load_w("k", wkT_ext)
                w_sb["v"] = load_w("v", wvT_ext)
                wo_all = wo_pool.tile([128, NPAIR * D], BF16, tag="wo",
                                      name="wo")
                nc.gpsimd.dma_start(
                    wo_all[:].rearrange("p (hdt c) -> p hdt c", c=D),
                    woT_ext[:].rearrange("(hdt p) c -> p hdt c", p=128))
                w_sb["o"] = [wo_all[:, hdt * D:(hdt + 1) * D]
                             for hdt in range(NPAIR)]
            act = (half == 0)
            q_units, kv_units = [], []
            for tb in tbs:
                for hdt in range(NPAIR):
                    q_units.append(
                        (lambda tb=tb, h=hdt:
                         qk_chain(tb, xts_all[tb], "q", QT, h, act)))
            for hdt in range(NPAIR):        # pair-major: KT[p] ready early
                for tb in tbs:
                    kv_units.append(
                        (lambda tb=tb, h=hdt:
                         qk_chain(tb, xts_all[tb], "k", KT, h, act)))
            for tb in tbs:
                for j in range(TB // 128):
                    kv_units.append(
                        (lambda tb=tb, j=j: v_chain(tb, xts_all[tb], j, act)))
            return deque(q_units), deque(kv_units)

        def scores_exp_pair(p, half, kt, pts):
            q0 = half * HALF
            off = max(0, kt * 128 - q0)
            pt = pt_pool.tile([128, 2 * HALF], BF16, tag=f"pt{kt % KGRP}",
                              name=f"pt{kt % KGRP}")
            pts[kt] = pt
            p3 = pt[:].rearrange("p (g q) -> p g q", q=HALF)
            for c in range(HALF // CHUNK):
                lo = c * CHUNK
                if lo + CHUNK <= off:
                    continue
                sub = max(0, off - lo)
                s = s_ps_pool.tile([128, 2 * CHUNK], F32, tag="s",
                                   name="sps")
                for h2 in range(2):
                    nc.tensor.matmul(
                        s[:, h2 * CHUNK + sub:(h2 + 1) * CHUNK],
                        KT[p][h2 * 64:h2 * 64 + 64,
                              kt * 128:(kt + 1) * 128],
                        QT[p][h2 * 64:h2 * 64 + 64,
                              q0 + lo + sub:q0 + lo + CHUNK],
                        start=True, stop=True)
                s3 = s[:].rearrange("p (g q) -> p g q", q=CHUNK)
                nc.scalar.activation(p3[:, :, lo + sub:lo + CHUNK],
                                     s3[:, :, sub:CHUNK], EXP, scale=0.125)
            if kt * 128 >= q0:
                nc.vector.tensor_mul(
                    p3[:, :, off:off + 128],
                    p3[:, :, off:off + 128],
                    mask_sb[:].rearrange("p (o q) -> p o q", o=1)
                    .broadcast_to([128, 2, 128]))

        def ctx_unit(p, half, h2, c, pts, ctxT_h, lbuf, first):
            """Context-chain over the kt's present in pts for chunk c."""
            q0 = half * HALF
            h = 2 * p + h2
            lo = c * CHUNK
            kts = [kt for kt in sorted(pts)
                   if max(0, kt * 128 - q0) < lo + CHUNK]
            if not kts:
                return
            cps = ctx_ps_pool.tile([65, CHUNK], F32, tag="ctx", name="ctxps")
            for i, kt in enumerate(kts):
                sub = max(0, max(0, kt * 128 - q0) - lo)
                nc.tensor.matmul(
                    cps[:, sub:CHUNK],
                    V[kt][:, h * 65:(h + 1) * 65],
                    pts[kt][:, h2 * HALF + lo + sub:h2 * HALF + lo + CHUNK],
                    start=(i == 0), stop=(i == len(kts) - 1),
                    skip_group_check=True)
            dst = ctxT_h[p][h2 * 64:h2 * 64 + 64, lo:lo + CHUNK]
            dden = lbuf[p][h2 * 64:h2 * 64 + 1, lo:lo + CHUNK]
            if first:
                nc.vector.tensor_copy(dst, cps[0:64, :])
                nc.vector.tensor_copy(dden, cps[64:65, :])
            else:
                nc.vector.tensor_add(dst, dst, cps[0:64, :])
                nc.vector.tensor_add(dden, dden, cps[64:65, :])

        def norm_pair_cc(ctxT_h, lbuf, p, cc):
            sl = slice(cc * CHUNK, (cc + 1) * CHUNK)
            nc.vector.reciprocal_approx_fast(lbuf[p][:, sl],
                                             lbuf[p][:, sl])
            for h2 in range(2):
                po = h2 * 64
                stage = norm_pool.tile([1, CHUNK], BF16, tag="stage",
                                       name="stage")
                nc.vector.tensor_copy(stage[:], lbuf[p][po:po + 1, sl])
                bc = norm_pool.tile([128, CHUNK], BF16, tag="bc",
                                    name="bc")
                nc.gpsimd.partition_broadcast(bc[:], stage[:])
                nc.vector.tensor_mul(
                    ctxT_h[p][po:po + 64, sl],
                    ctxT_h[p][po:po + 64, sl],
                    bc[po:po + 64, :])

        def outproj_units(half, ctxT_h):
            q0 = half * HALF
            units = []

            def one(tt, ob):
                ps = mm_ps_pool.tile([128, CHUNK], F32, tag="mm", name="opps")
                for p in range(NPAIR):
                    nc.tensor.matmul(
                        ps[:],
                        ctxT_h[p][:, tt * 128:(tt + 1) * 128],
                        w_sb["o"][p][:, ob * CHUNK:(ob + 1) * CHUNK],
                        start=(p == 0), stop=(p == NPAIR - 1))
                osb = out_pool.tile([128, CHUNK], F32, tag="osb", name="osb")
                nc.vector.tensor_add(
                    osb[:], ps[:], bias_bc[:, ob * CHUNK:(ob + 1) * CHUNK])
                nc.sync.dma_start(
                    out_ext[q0 + tt * 128:q0 + (tt + 1) * 128,
                            ob * CHUNK:(ob + 1) * CHUNK],
                    osb[:])

            for tt in range(HALF // 128):
                for ob in range(D // CHUNK):
                    units.append(lambda tt=tt, ob=ob: one(tt, ob))
            return deque(units)

        def attn_half(half, fill):
            aux = deque()
            q0 = half * HALF
            nkt = (q0 + HALF) // 128
            ctxT_h = [ctxT_pool.tile([128, HALF], BF16, tag=f"ctxT{i}",
                                     name=f"ctxT{i}")
                      for i in range(NPAIR)]
            lbuf = [lbuf_pool.tile([65, HALF], F32, tag=f"lb{j}",
                                   name=f"lb{j}")
                    for j in range(NPAIR)]
            for p in range(NPAIR):
                for g in range(nkt // KGRP):
                    pts = {}
                    for kt in range(g * KGRP, (g + 1) * KGRP):
                        scores_exp_pair(p, half, kt, pts)
                        if DBG and half == 0 and p == 0 and kt == 0:
                            nc.sync.dma_start(dbg_pt[:], pts[0][:])
                        if aux:
                            aux.popleft()()
                        if fill:
                            fill.popleft()()
                    for c in range(HALF // CHUNK):
                        for h2 in range(2):
                            ctx_unit(p, half, h2, c, pts, ctxT_h, lbuf,
                                     first=(g == 0))
                for cc in range(HALF // CHUNK):
                    aux.append(lambda p=p, cc=cc:
                               norm_pair_cc(ctxT_h, lbuf, p, cc))
            while aux:
                aux.popleft()()
            if DBG and half == 0:
                nc.sync.dma_start(dbg_lb[:], lbuf[0][:])
            if DBG and half == 0:
                nc.sync.dma_start(dbg_ctx[:], ctxT_h[0][:])
                nc.sync.dma_start(dbg_q[:], QT[0][:])
                nc.sync.dma_start(dbg_k[:], KT[0][:])
            return ctxT_h

        # ---------------- emission schedule ----------------------------
        q0u, kv0u = qkv_units(0)
        while q0u:
            q0u.popleft()()
        while kv0u:
            kv0u.popleft()()

        q1u, kv1u = qkv_units(1)        # emits x(h1) load DMAs up front
        ctxT_0 = attn_half(0, q1u)      # Q(h1) chains fill attn(h0)
        while q1u:
            q1u.popleft()()

        fill1 = kv1u                    # K/V(h1) chains fill attn(h1) grp 0
        ctxT_1_box = []

        def run_attn1():
            # outproj(h0) chunks join the fill queue once norm(h0) is done
            fill1.extend(outproj_units(0, ctxT_0))
            ctxT_1_box.append(attn_half(1, fill1))

        run_attn1()
        while fill1:
            fill1.popleft()()

        op1 = outproj_units(1, ctxT_1_box[0])
        while op1:
            op1.popleft()()

    nc.compile()
    return nc


def _get_nc():
    global _NC
    if _NC is None:
        _NC = _build()
    return _NC


def _make_in_maps(x, Wq, Wk, Wv, Wo, bo):
    import ml_dtypes
    mask = np.triu(np.ones((128, 128), dtype=np.float32)).astype(
        ml_dtypes.bfloat16)
    zero_bias = np.zeros((1, D), dtype=np.float32)
    xT = [np.ascontiguousarray(x[b].T) for b in range(B)]
    in_maps = []
    for c in range(N_CORES):
        b, g = c // 2, c % 2
        sl = slice(g * HD, (g + 1) * HD)
        in_maps.append({
            "xT": xT[b].astype(ml_dtypes.bfloat16),
            "wqT": np.ascontiguousarray(Wq[sl, :].T).astype(ml_dtypes.bfloat16),
            "wkT": np.ascontiguousarray(Wk[sl, :].T).astype(ml_dtypes.bfloat16),
            "wvT": np.ascontiguousarray(Wv[sl, :].T).astype(ml_dtypes.bfloat16),
            "woT": np.ascontiguousarray(Wo[:, sl].T).astype(ml_dtypes.bfloat16),
            "bias": bo.reshape(1, D) if g == 0 else zero_bias,
            "mask": mask,
        })
    return in_maps


def kernel(x, Wq, Wk, Wv, Wo, bo):
    from concourse.bass_utils import run_bass_kernel_spmd

    x = np.asarray(x, dtype=np.float32)
    Wq = np.asarray(Wq, dtype=np.float32)
    Wk = np.asarray(Wk, dtype=np.float32)
    Wv = np.asarray(Wv, dtype=np.float32)
    Wo = np.asarray(Wo, dtype=np.float32)
    bo = np.asarray(bo, dtype=np.float32)

    nc = _get_nc()
    in_maps = _make_in_maps(x, Wq, Wk, Wv, Wo, bo)
    res = run_bass_kernel_spmd(nc, in_maps, list(range(N_CORES)))
    outs = [res.results[c]["out"] for c in range(N_CORES)]
    return np.stack([outs[2 * b] + outs[2 * b + 1] for b in range(B)], axis=0)
